# revision 30
# baseline (speedup 1.0000x reference)
"""Additive (Bahdanau) attention on 8 TRN2 NeuronCores.

Data-parallel over the batch dim: each of the 8 cores gets B/8 = 4 batch rows.
Per core, per batch row b:
    scores[l] = v . tanh(w@query + w_b + u_b + memory[l] @ u_W.T)   (v_b dropped:
                softmax is shift-invariant, so it cancels in both outputs)
    weights   = softmax(scores + log(mask))  (implemented as exp(s-max)*mask / Z)
    aggr      = weights @ memory

Layout trick: scores are computed TRANSPOSED (scoresT[l % 128 partitions,
l // 128 free]) by using the tanh-values as the stationary matmul operand, so
softmax runs as wide [128, L/128] ops and the softmax weights are already in
the [l_partition, 1] orientation the aggregation matmul needs as lhsT.

The big memory tensor is passed from host in BOTH layouts (transposed for the
scores matmul whose contraction is over the feature dim, natural for the
aggregation whose contraction is over L), cast to bf16 on host.  HBM traffic
per core = 2 * 32 MB = 64 MB, the same bytes as one f32 read of memory.
"""

import sys

for _p in ("/opt/trn_rl_repo",):
    if _p not in sys.path:
        sys.path.insert(0, _p)

import numpy as np
import ml_dtypes

BF16 = ml_dtypes.bfloat16

# Problem shape (hardcoded per the task spec)
B, L = 32, 4096
Q_SIZE, M_SIZE, A_SIZE = 1024, 1024, 256
N_CORES = 8
BPC = B // N_CORES  # batch rows per core


def build_nc(bpc=BPC, l_size=L, m_size=M_SIZE, a_size=A_SIZE, q_size=Q_SIZE,
             t_bufs=3, n_bufs=4, um_bufs=4, stages=7):
    """Build the single-core Bass graph (run SPMD on all 8 cores)."""
    import concourse.bass as bass
    import concourse.tile as tile
    from concourse import bacc, mybir, bass_isa

    f32 = mybir.dt.float32
    bf16 = mybir.dt.bfloat16
    AF = mybir.ActivationFunctionType
    Alu = mybir.AluOpType

    LT = min(512, l_size)         # l-tile size for the scores pass
    n_lt = l_size // LT           # l-tiles per batch row
    n_mc = m_size // 128          # 128-contraction chunks of the feature dim
    n_qc = q_size // 128
    n_ah = a_size // 128          # attention-dim halves (psum partition chunks)
    n_lc = l_size // 128          # l-chunks (aggr contraction)
    NG = min(1024, l_size)        # natural-layout l rows per DMA group
    n_ng = l_size // NG
    AGC = min(512, m_size)        # aggr output free-dim chunk
    n_mh = m_size // AGC

    nc = bacc.Bacc("TRN2", target_bir_lowering=False, debug=False)

    ones_d = nc.declare_dram_parameter("ones_d", [128], f32, isOutput=False)
    memT = nc.declare_dram_parameter("memT", [bpc, m_size, l_size], bf16, isOutput=False)
    memN = nc.declare_dram_parameter("memN", [bpc, l_size, m_size], bf16, isOutput=False)
    uWT = nc.declare_dram_parameter("uWT", [m_size, a_size], bf16, isOutput=False)
    wWT = nc.declare_dram_parameter("wWT", [q_size, a_size], bf16, isOutput=False)
    qT = nc.declare_dram_parameter("qT", [q_size, bpc], bf16, isOutput=False)
    bias_c = nc.declare_dram_parameter("bias_c", [a_size], f32, isOutput=False)
    v_col = nc.declare_dram_parameter("v_col", [a_size], bf16, isOutput=False)
    maskT = nc.declare_dram_parameter("maskT", [bpc, 128, n_lc], f32, isOutput=False)
    ident = nc.declare_dram_parameter("ident", [128, 128], f32, isOutput=False)

    out_aggr = nc.declare_dram_parameter("out_aggr", [bpc, m_size], f32, isOutput=True)
    out_w = nc.declare_dram_parameter("out_w", [bpc, l_size], f32, isOutput=True)

    with tile.TileContext(nc) as tc:
        with (
            tc.tile_pool(name="const", bufs=1) as cpool,
            tc.tile_pool(name="ttile", bufs=t_bufs) as tpool,
            tc.tile_pool(name="ntile", bufs=n_bufs) as npool,
            tc.tile_pool(name="tanh", bufs=4) as thpool,
            tc.tile_pool(name="small", bufs=2) as spool,
            tc.tile_pool(name="psum_um", bufs=um_bufs, space=bass.MemorySpace.PSUM) as pum,
            tc.tile_pool(name="psum_misc", bufs=2, space=bass.MemorySpace.PSUM) as pmisc,
            tc.tile_pool(name="psum_aggr", bufs=2, space=bass.MemorySpace.PSUM) as pagg,
        ):
            # ---- constants ----
            uwt = cpool.tile([128, n_mc, a_size], bf16)
            nc.sync.dma_start(uwt[:], uWT.rearrange("(c p) a -> p c a", p=128))
            wwt = cpool.tile([128, n_qc, a_size], bf16)
            nc.sync.dma_start(wwt[:], wWT.rearrange("(c p) a -> p c a", p=128))
            qt = cpool.tile([128, n_qc, bpc], bf16)
            nc.sync.dma_start(qt[:], qT.rearrange("(c p) b -> p c b", p=128))
            bc = cpool.tile([128, n_ah], f32)
            nc.sync.dma_start(bc[:], bias_c.rearrange("(h p) -> p h", p=128))
            vc = cpool.tile([128, n_ah], bf16)
            nc.sync.dma_start(vc[:], v_col.rearrange("(h p) -> p h", p=128))
            idt = cpool.tile([128, 128], f32)
            nc.sync.dma_start(idt[:], ident[:])
            ones_r = cpool.tile([1, 128], f32)
            nc.sync.dma_start(ones_r[:], ones_d[None, :])

            def cross_part_reduce(vec, op, nm):
                """All-partition reduce of a [128, 1] f32 SBUF vec -> [128, 1]
                broadcast, via PE transpose + free-dim reduce + ones-matmul."""
                tp = pmisc.tile([1, 128], f32, tag="misc", name=f"{nm}_tp")
                nc.tensor.transpose(tp[:], vec[:], idt[:])
                ts = spool.tile([1, 128], f32, tag=f"{nm}_ts", name=f"{nm}_ts")
                nc.vector.tensor_copy(ts[:], tp[:])
                r = spool.tile([1, 1], f32, tag=f"{nm}_r", name=f"{nm}_r")
                nc.vector.tensor_reduce(r[:], ts[:], op=op,
                                        axis=mybir.AxisListType.X)
                bp = pmisc.tile([128, 1], f32, tag="misc", name=f"{nm}_bp")
                nc.tensor.matmul(bp[:], ones_r[:], r[:], start=True, stop=True)
                bs = spool.tile([128, 1], f32, tag=f"{nm}_bs", name=f"{nm}_bs")
                nc.vector.tensor_copy(bs[:], bp[:])
                return bs

            # ---- wq: cT[a, b] = sum_q w_W[a, q] query[b, q]  (+ w_b + u_b) ----
            ct = cpool.tile([128, n_ah, bpc], f32)
            for ah in range(n_ah):
                cps = pmisc.tile([128, bpc], f32, tag="misc")
                for qc in range(n_qc):
                    nc.tensor.matmul(
                        cps[:],
                        wwt[:, qc, ah * 128:(ah + 1) * 128],
                        qt[:, qc, :],
                        start=(qc == 0), stop=(qc == n_qc - 1),
                    )
                # ScalarE add: TensorScalarPtr (DVE) allows only ONE sync-wait
                # slot and this op needs two (PE matmul + bias DMA).
                nc.scalar.add(ct[:, ah, :], cps[:], bc[:, ah:ah + 1])

            # ---- main loop over batch rows ----
            for b in range(bpc):
                st = spool.tile([128, n_lc], f32, tag="scoresT")
                mk = spool.tile([128, n_lc], f32, tag="maskT")
                nc.sync.dma_start(mk[:], maskT[b])

                # --- scores pass: stream transposed-layout tiles ---
                if stages < 1:
                    continue
                for t in range(n_lt):
                    tt = tpool.tile([128, n_mc, LT], bf16)
                    nc.sync.dma_start(
                        tt[:],
                        memT[b, :, t * LT:(t + 1) * LT].rearrange(
                            "(c p) j -> p c j", p=128),
                    )
                    sps = pmisc.tile([128, LT // 128], f32, tag="misc")
                    ths = []
                    for ah in range(n_ah):
                        ups = pum.tile([128, LT], f32)
                        for mc in range(n_mc):
                            nc.tensor.matmul(
                                ups[:],
                                uwt[:, mc, ah * 128:(ah + 1) * 128],
                                tt[:, mc, :],
                                start=(mc == 0), stop=(mc == n_mc - 1),
                            )
                        th = thpool.tile([128, LT], bf16, name=f"th{ah}",
                                         tag="th")
                        nc.scalar.activation(th[:], ups[:], AF.Tanh,
                                             bias=ct[:, ah, b:b + 1])
                        ths.append(th)
                    if stages < 2:
                        continue
                    # scoresT: lhsT = tanh block (stationary), rhs = v col;
                    # each sps column's accumulation group must complete before
                    # the next column's start (start=True clears the whole
                    # bank's has_written bits).
                    for ls in range(LT // 128):
                        for ah in range(n_ah):
                            nc.tensor.matmul(
                                sps[:, ls:ls + 1],
                                ths[ah][:, ls * 128:(ls + 1) * 128],
                                vc[:, ah:ah + 1],
                                start=(ah == 0), stop=(ah == n_ah - 1),
                            )
                    nc.vector.tensor_copy(
                        st[:, t * (LT // 128):(t + 1) * (LT // 128)], sps[:])

                if stages < 3:
                    continue
                # --- softmax over l (transposed layout [128, n_lc]) ---
                mx = spool.tile([128, 1], f32, tag="mx")
                nc.vector.reduce_max(mx[:], st[:], axis=mybir.AxisListType.X)
                mxr = cross_part_reduce(mx, Alu.max, "mxr")
                negm = spool.tile([128, 1], f32, tag="negm")
                nc.vector.tensor_scalar_mul(negm[:], mxr[:], -1.0)
                et = spool.tile([128, n_lc], f32, tag="expT")
                nc.scalar.activation(et[:], st[:], AF.Exp, bias=negm[:])
                # (tensor_tensor_reduce is an ANT-custom DVE op the runtime
                # here can't load; use standard mult + reduce instead)
                pu = spool.tile([128, n_lc], f32, tag="puT")
                nc.vector.tensor_mul(pu[:], et[:], mk[:])
                zp = spool.tile([128, 1], f32, tag="zpart")
                nc.vector.reduce_sum(zp[:], pu[:], axis=mybir.AxisListType.X)
                z = cross_part_reduce(zp, Alu.add, "z")
                rz = spool.tile([128, 1], f32, tag="rz")
                nc.vector.reciprocal(rz[:], z[:])
                pn = spool.tile([128, n_lc], f32, tag="pnT")
                nc.vector.tensor_scalar_mul(pn[:], pu[:], rz[:])
                pnb = spool.tile([128, n_lc], bf16, tag="pnTb")
                nc.vector.tensor_copy(pnb[:], pn[:])

                if stages < 4:
                    continue
                # --- weights output: transpose pn back to natural layout ---
                wps = pmisc.tile([128, 128], f32, tag="misc")
                nc.tensor.transpose(wps[:n_lc, :], pn[:], idt[:])
                wsb = spool.tile([128, 128], f32, tag="wsb")
                nc.vector.tensor_copy(wsb[:n_lc, :], wps[:n_lc, :])
                nc.scalar.dma_start(
                    out_w[b].rearrange("(c p) -> c p", p=128), wsb[:n_lc, :])

                if stages < 5:
                    continue
                # --- aggregation pass: stream natural-layout tiles ---
                aps = [pagg.tile([1, AGC], f32, tag="aps", name=f"aps{mh}")
                       for mh in range(n_mh)]
                for g in range(n_ng):
                    nt = npool.tile([128, NG // 128, m_size], bf16)
                    nc.sync.dma_start(
                        nt[:],
                        memN[b, g * NG:(g + 1) * NG, :].rearrange(
                            "(gg p) m -> p gg m", p=128),
                    )
                    if stages < 6:
                        continue
                    for gg in range(NG // 128):
                        c = g * (NG // 128) + gg
                        for mh in range(n_mh):
                            nc.tensor.matmul(
                                aps[mh][:],
                                pnb[:, c:c + 1],
                                nt[:, gg, mh * AGC:(mh + 1) * AGC],
                                start=(c == 0), stop=(c == n_lc - 1),
                            )
                if stages < 7:
                    continue
                asb = spool.tile([1, m_size], f32, tag="asb")
                for mh in range(n_mh):
                    nc.vector.tensor_copy(asb[:, mh * AGC:(mh + 1) * AGC],
                                          aps[mh][:])
                nc.scalar.dma_start(out_aggr[b:b + 1, :], asb[:])

    nc.compile()
    return nc


def _prep_in_maps(query, memory, memory_mask, w_W, w_b, u_W, u_b, v_W, v_b,
                  n_cores=N_CORES):
    bpc = memory.shape[0] // n_cores
    l_size = memory.shape[1]
    n_lc = l_size // 128
    uWT = np.ascontiguousarray(u_W.T).astype(BF16)
    wWT = np.ascontiguousarray(w_W.T).astype(BF16)
    bias_c = (w_b + u_b).astype(np.float32)
    v_col = v_W[0].astype(BF16)
    ident = np.eye(128, dtype=np.float32)
    ones_d = np.ones(128, dtype=np.float32)
    in_maps = []
    for i in range(n_cores):
        b0 = i * bpc
        mem = memory[b0:b0 + bpc]
        in_maps.append({
            "memT": mem.transpose(0, 2, 1).astype(BF16),
            "memN": mem.astype(BF16),
            "uWT": uWT,
            "wWT": wWT,
            "qT": np.ascontiguousarray(query[b0:b0 + bpc].T).astype(BF16),
            "bias_c": bias_c,
            "v_col": v_col,
            "maskT": np.ascontiguousarray(
                memory_mask[b0:b0 + bpc].reshape(bpc, n_lc, 128)
                .transpose(0, 2, 1)).astype(np.float32),
            "ident": ident,
            "ones_d": ones_d,
        })
    return in_maps


_NC_CACHE = {}


def _run(inputs, trace=False, **trace_kw):
    from concourse.bass_utils import run_bass_kernel_spmd

    in_maps = _prep_in_maps(**inputs)
    if "nc" not in _NC_CACHE:
        _NC_CACHE["nc"] = build_nc()
    nc = _NC_CACHE["nc"]
    res = run_bass_kernel_spmd(nc, in_maps, core_ids=list(range(N_CORES)),
                               trace=trace, **trace_kw)
    aggr = np.concatenate([r["out_aggr"] for r in res.results], axis=0)
    weights = np.concatenate([r["out_w"] for r in res.results], axis=0)
    return (aggr.astype(np.float32), weights.astype(np.float32)), res


def kernel(query, memory, memory_mask, w_W, w_b, u_W, u_b, v_W, v_b):
    out, _ = _run(dict(query=query, memory=memory, memory_mask=memory_mask,
                       w_W=w_W, w_b=w_b, u_W=u_W, u_b=u_b, v_W=v_W, v_b=v_b))
    return out


if __name__ == "__main__":
    rng = np.random.default_rng(0)
    inputs = {
        "query": rng.standard_normal((B, Q_SIZE), dtype=np.float32),
        "memory": rng.standard_normal((B, L, M_SIZE), dtype=np.float32),
        "memory_mask": np.ones((B, L), dtype=np.float32),
        "w_W": rng.standard_normal((A_SIZE, Q_SIZE), dtype=np.float32) / 32,
        "w_b": rng.standard_normal(A_SIZE, dtype=np.float32) * 0.01,
        "u_W": rng.standard_normal((A_SIZE, M_SIZE), dtype=np.float32) / 32,
        "u_b": rng.standard_normal(A_SIZE, dtype=np.float32) * 0.01,
        "v_W": rng.standard_normal((1, A_SIZE), dtype=np.float32) / 16,
        "v_b": rng.standard_normal(1, dtype=np.float32) * 0.01,
    }
    out = kernel(**inputs)
    print([o.shape for o in out])


# revision 31
# speedup vs baseline: 1.2149x; 1.2149x over previous
"""Additive (Bahdanau) attention on 8 TRN2 NeuronCores.

Data-parallel over the batch dim: each of the 8 cores gets B/8 = 4 batch rows.
Per core, per batch row b:
    scores[l] = v . tanh(w@query + w_b + u_b + memory[l] @ u_W.T)   (v_b dropped:
                softmax is shift-invariant, so it cancels in both outputs)
    weights   = softmax(scores + log(mask))  (implemented as exp(s-max)*mask / Z)
    aggr      = weights @ memory

Layout trick: scores are computed TRANSPOSED (scoresT[l % 128 partitions,
l // 128 free]) by using the tanh-values as the stationary matmul operand, so
softmax runs as wide [128, L/128] ops and the softmax weights are already in
the [l_partition, 1] orientation the aggregation matmul needs as lhsT.

The big memory tensor is passed from host in BOTH layouts (transposed for the
scores matmul whose contraction is over the feature dim, natural for the
aggregation whose contraction is over L), cast to bf16 on host.  HBM traffic
per core = 2 * 32 MB = 64 MB, the same bytes as one f32 read of memory.
"""

import sys

for _p in ("/opt/trn_rl_repo",):
    if _p not in sys.path:
        sys.path.insert(0, _p)

import numpy as np
import ml_dtypes

BF16 = ml_dtypes.bfloat16

# Problem shape (hardcoded per the task spec)
B, L = 32, 4096
Q_SIZE, M_SIZE, A_SIZE = 1024, 1024, 256
N_CORES = 8
BPC = B // N_CORES  # batch rows per core


def build_nc(bpc=BPC, l_size=L, m_size=M_SIZE, a_size=A_SIZE, q_size=Q_SIZE,
             t_bufs=6, n_bufs=6, um_bufs=4, stages=7):
    """Build the single-core Bass graph (run SPMD on all 8 cores)."""
    import concourse.bass as bass
    import concourse.tile as tile
    from concourse import bacc, mybir, bass_isa

    f32 = mybir.dt.float32
    bf16 = mybir.dt.bfloat16
    AF = mybir.ActivationFunctionType
    Alu = mybir.AluOpType

    LT = min(512, l_size)         # l-tile size for the scores pass
    n_lt = l_size // LT           # l-tiles per batch row
    n_mc = m_size // 128          # 128-contraction chunks of the feature dim
    n_qc = q_size // 128
    n_ah = a_size // 128          # attention-dim halves (psum partition chunks)
    n_lc = l_size // 128          # l-chunks (aggr contraction)
    NG = min(1024, l_size)        # natural-layout l rows per DMA group
    n_ng = l_size // NG
    AGC = min(512, m_size)        # aggr output free-dim chunk
    n_mh = m_size // AGC

    nc = bacc.Bacc("TRN2", target_bir_lowering=False, debug=False)

    ones_d = nc.declare_dram_parameter("ones_d", [128], f32, isOutput=False)
    memT = nc.declare_dram_parameter("memT", [bpc, m_size, l_size], bf16, isOutput=False)
    memN = nc.declare_dram_parameter("memN", [bpc, l_size, m_size], bf16, isOutput=False)
    uWT = nc.declare_dram_parameter("uWT", [m_size, a_size], bf16, isOutput=False)
    wWT = nc.declare_dram_parameter("wWT", [q_size, a_size], bf16, isOutput=False)
    qT = nc.declare_dram_parameter("qT", [q_size, bpc], bf16, isOutput=False)
    bias_c = nc.declare_dram_parameter("bias_c", [a_size], f32, isOutput=False)
    v_col = nc.declare_dram_parameter("v_col", [a_size], bf16, isOutput=False)
    maskT = nc.declare_dram_parameter("maskT", [bpc, 128, n_lc], f32, isOutput=False)
    ident = nc.declare_dram_parameter("ident", [128, 128], f32, isOutput=False)

    out_aggr = nc.declare_dram_parameter("out_aggr", [bpc, m_size], f32, isOutput=True)
    out_w = nc.declare_dram_parameter("out_w", [bpc, l_size], f32, isOutput=True)

    with tile.TileContext(nc) as tc:
        with (
            tc.tile_pool(name="const", bufs=1) as cpool,
            tc.tile_pool(name="ttile", bufs=t_bufs) as tpool,
            tc.tile_pool(name="ntile", bufs=n_bufs) as npool,
            tc.tile_pool(name="tanh", bufs=4) as thpool,
            tc.tile_pool(name="small", bufs=2) as spool,
            tc.tile_pool(name="psum_um", bufs=um_bufs, space=bass.MemorySpace.PSUM) as pum,
            tc.tile_pool(name="psum_misc", bufs=2, space=bass.MemorySpace.PSUM) as pmisc,
            tc.tile_pool(name="psum_aggr", bufs=2, space=bass.MemorySpace.PSUM) as pagg,
        ):
            # ---- constants ----
            uwt = cpool.tile([128, n_mc, a_size], bf16)
            nc.sync.dma_start(uwt[:], uWT.rearrange("(c p) a -> p c a", p=128))
            wwt = cpool.tile([128, n_qc, a_size], bf16)
            nc.sync.dma_start(wwt[:], wWT.rearrange("(c p) a -> p c a", p=128))
            qt = cpool.tile([128, n_qc, bpc], bf16)
            nc.sync.dma_start(qt[:], qT.rearrange("(c p) b -> p c b", p=128))
            bc = cpool.tile([128, n_ah], f32)
            nc.sync.dma_start(bc[:], bias_c.rearrange("(h p) -> p h", p=128))
            vc = cpool.tile([128, n_ah], bf16)
            nc.sync.dma_start(vc[:], v_col.rearrange("(h p) -> p h", p=128))
            idt = cpool.tile([128, 128], f32)
            nc.sync.dma_start(idt[:], ident[:])
            ones_r = cpool.tile([1, 128], f32)
            nc.sync.dma_start(ones_r[:], ones_d[None, :])

            def cross_part_reduce(vec, op, nm):
                """All-partition reduce of a [128, 1] f32 SBUF vec -> [128, 1]
                broadcast, via PE transpose + free-dim reduce + ones-matmul."""
                tp = pmisc.tile([1, 128], f32, tag="misc", name=f"{nm}_tp")
                nc.tensor.transpose(tp[:], vec[:], idt[:])
                ts = spool.tile([1, 128], f32, tag=f"{nm}_ts", name=f"{nm}_ts")
                nc.vector.tensor_copy(ts[:], tp[:])
                r = spool.tile([1, 1], f32, tag=f"{nm}_r", name=f"{nm}_r")
                nc.vector.tensor_reduce(r[:], ts[:], op=op,
                                        axis=mybir.AxisListType.X)
                bp = pmisc.tile([128, 1], f32, tag="misc", name=f"{nm}_bp")
                nc.tensor.matmul(bp[:], ones_r[:], r[:], start=True, stop=True)
                bs = spool.tile([128, 1], f32, tag=f"{nm}_bs", name=f"{nm}_bs")
                nc.vector.tensor_copy(bs[:], bp[:])
                return bs

            # ---- wq: cT[a, b] = sum_q w_W[a, q] query[b, q]  (+ w_b + u_b) ----
            ct = cpool.tile([128, n_ah, bpc], f32)
            for ah in range(n_ah):
                cps = pmisc.tile([128, bpc], f32, tag="misc")
                for qc in range(n_qc):
                    nc.tensor.matmul(
                        cps[:],
                        wwt[:, qc, ah * 128:(ah + 1) * 128],
                        qt[:, qc, :],
                        start=(qc == 0), stop=(qc == n_qc - 1),
                    )
                # ScalarE add: TensorScalarPtr (DVE) allows only ONE sync-wait
                # slot and this op needs two (PE matmul + bias DMA).
                nc.scalar.add(ct[:, ah, :], cps[:], bc[:, ah:ah + 1])

            # ---- main loop over batch rows ----
            for b in range(bpc):
                st = spool.tile([128, n_lc], f32, tag="scoresT")
                mk = spool.tile([128, n_lc], f32, tag="maskT")
                nc.sync.dma_start(mk[:], maskT[b])

                # --- scores pass: stream transposed-layout tiles ---
                if stages < 1:
                    continue
                for t in range(n_lt):
                    tt = tpool.tile([128, n_mc, LT], bf16)
                    nc.sync.dma_start(
                        tt[:],
                        memT[b, :, t * LT:(t + 1) * LT].rearrange(
                            "(c p) j -> p c j", p=128),
                    )
                    sps = pmisc.tile([128, LT // 128], f32, tag="misc")
                    ths = []
                    for ah in range(n_ah):
                        ups = pum.tile([128, LT], f32)
                        for mc in range(n_mc):
                            nc.tensor.matmul(
                                ups[:],
                                uwt[:, mc, ah * 128:(ah + 1) * 128],
                                tt[:, mc, :],
                                start=(mc == 0), stop=(mc == n_mc - 1),
                            )
                        th = thpool.tile([128, LT], bf16, name=f"th{ah}",
                                         tag="th")
                        nc.scalar.activation(th[:], ups[:], AF.Tanh,
                                             bias=ct[:, ah, b:b + 1])
                        ths.append(th)
                    if stages < 2:
                        continue
                    # scoresT: lhsT = tanh block (stationary), rhs = v col;
                    # each sps column's accumulation group must complete before
                    # the next column's start (start=True clears the whole
                    # bank's has_written bits).
                    for ls in range(LT // 128):
                        for ah in range(n_ah):
                            nc.tensor.matmul(
                                sps[:, ls:ls + 1],
                                ths[ah][:, ls * 128:(ls + 1) * 128],
                                vc[:, ah:ah + 1],
                                start=(ah == 0), stop=(ah == n_ah - 1),
                            )
                    nc.vector.tensor_copy(
                        st[:, t * (LT // 128):(t + 1) * (LT // 128)], sps[:])

                if stages < 3:
                    continue
                # --- softmax over l (transposed layout [128, n_lc]) ---
                mx = spool.tile([128, 1], f32, tag="mx")
                nc.vector.reduce_max(mx[:], st[:], axis=mybir.AxisListType.X)
                mxr = cross_part_reduce(mx, Alu.max, "mxr")
                negm = spool.tile([128, 1], f32, tag="negm")
                nc.vector.tensor_scalar_mul(negm[:], mxr[:], -1.0)
                et = spool.tile([128, n_lc], f32, tag="expT")
                nc.scalar.activation(et[:], st[:], AF.Exp, bias=negm[:])
                # (tensor_tensor_reduce is an ANT-custom DVE op the runtime
                # here can't load; use standard mult + reduce instead)
                pu = spool.tile([128, n_lc], f32, tag="puT")
                nc.vector.tensor_mul(pu[:], et[:], mk[:])
                zp = spool.tile([128, 1], f32, tag="zpart")
                nc.vector.reduce_sum(zp[:], pu[:], axis=mybir.AxisListType.X)
                z = cross_part_reduce(zp, Alu.add, "z")
                rz = spool.tile([128, 1], f32, tag="rz")
                nc.vector.reciprocal(rz[:], z[:])
                pn = spool.tile([128, n_lc], f32, tag="pnT")
                nc.vector.tensor_scalar_mul(pn[:], pu[:], rz[:])
                pnb = spool.tile([128, n_lc], bf16, tag="pnTb")
                nc.vector.tensor_copy(pnb[:], pn[:])

                if stages < 4:
                    continue
                # --- weights output: transpose pn back to natural layout ---
                wps = pmisc.tile([128, 128], f32, tag="misc")
                nc.tensor.transpose(wps[:n_lc, :], pn[:], idt[:])
                wsb = spool.tile([128, 128], f32, tag="wsb")
                nc.vector.tensor_copy(wsb[:n_lc, :], wps[:n_lc, :])
                nc.scalar.dma_start(
                    out_w[b].rearrange("(c p) -> c p", p=128), wsb[:n_lc, :])

                if stages < 5:
                    continue
                # --- aggregation pass: stream natural-layout tiles ---
                aps = [pagg.tile([1, AGC], f32, tag="aps", name=f"aps{mh}")
                       for mh in range(n_mh)]
                for g in range(n_ng):
                    nt = npool.tile([128, NG // 128, m_size], bf16)
                    nc.sync.dma_start(
                        nt[:],
                        memN[b, g * NG:(g + 1) * NG, :].rearrange(
                            "(gg p) m -> p gg m", p=128),
                    )
                    if stages < 6:
                        continue
                    for gg in range(NG // 128):
                        c = g * (NG // 128) + gg
                        for mh in range(n_mh):
                            nc.tensor.matmul(
                                aps[mh][:],
                                pnb[:, c:c + 1],
                                nt[:, gg, mh * AGC:(mh + 1) * AGC],
                                start=(c == 0), stop=(c == n_lc - 1),
                            )
                if stages < 7:
                    continue
                asb = spool.tile([1, m_size], f32, tag="asb")
                for mh in range(n_mh):
                    nc.vector.tensor_copy(asb[:, mh * AGC:(mh + 1) * AGC],
                                          aps[mh][:])
                nc.scalar.dma_start(out_aggr[b:b + 1, :], asb[:])

    nc.compile()
    return nc


def _prep_in_maps(query, memory, memory_mask, w_W, w_b, u_W, u_b, v_W, v_b,
                  n_cores=N_CORES):
    bpc = memory.shape[0] // n_cores
    l_size = memory.shape[1]
    n_lc = l_size // 128
    uWT = np.ascontiguousarray(u_W.T).astype(BF16)
    wWT = np.ascontiguousarray(w_W.T).astype(BF16)
    bias_c = (w_b + u_b).astype(np.float32)
    v_col = v_W[0].astype(BF16)
    ident = np.eye(128, dtype=np.float32)
    ones_d = np.ones(128, dtype=np.float32)
    in_maps = []
    for i in range(n_cores):
        b0 = i * bpc
        mem = memory[b0:b0 + bpc]
        in_maps.append({
            "memT": mem.transpose(0, 2, 1).astype(BF16),
            "memN": mem.astype(BF16),
            "uWT": uWT,
            "wWT": wWT,
            "qT": np.ascontiguousarray(query[b0:b0 + bpc].T).astype(BF16),
            "bias_c": bias_c,
            "v_col": v_col,
            "maskT": np.ascontiguousarray(
                memory_mask[b0:b0 + bpc].reshape(bpc, n_lc, 128)
                .transpose(0, 2, 1)).astype(np.float32),
            "ident": ident,
            "ones_d": ones_d,
        })
    return in_maps


_NC_CACHE = {}


def _run(inputs, trace=False, **trace_kw):
    from concourse.bass_utils import run_bass_kernel_spmd

    in_maps = _prep_in_maps(**inputs)
    if "nc" not in _NC_CACHE:
        _NC_CACHE["nc"] = build_nc()
    nc = _NC_CACHE["nc"]
    res = run_bass_kernel_spmd(nc, in_maps, core_ids=list(range(N_CORES)),
                               trace=trace, **trace_kw)
    aggr = np.concatenate([r["out_aggr"] for r in res.results], axis=0)
    weights = np.concatenate([r["out_w"] for r in res.results], axis=0)
    return (aggr.astype(np.float32), weights.astype(np.float32)), res


def kernel(query, memory, memory_mask, w_W, w_b, u_W, u_b, v_W, v_b):
    out, _ = _run(dict(query=query, memory=memory, memory_mask=memory_mask,
                       w_W=w_W, w_b=w_b, u_W=u_W, u_b=u_b, v_W=v_W, v_b=v_b))
    return out


if __name__ == "__main__":
    rng = np.random.default_rng(0)
    inputs = {
        "query": rng.standard_normal((B, Q_SIZE), dtype=np.float32),
        "memory": rng.standard_normal((B, L, M_SIZE), dtype=np.float32),
        "memory_mask": np.ones((B, L), dtype=np.float32),
        "w_W": rng.standard_normal((A_SIZE, Q_SIZE), dtype=np.float32) / 32,
        "w_b": rng.standard_normal(A_SIZE, dtype=np.float32) * 0.01,
        "u_W": rng.standard_normal((A_SIZE, M_SIZE), dtype=np.float32) / 32,
        "u_b": rng.standard_normal(A_SIZE, dtype=np.float32) * 0.01,
        "v_W": rng.standard_normal((1, A_SIZE), dtype=np.float32) / 16,
        "v_b": rng.standard_normal(1, dtype=np.float32) * 0.01,
    }
    out = kernel(**inputs)
    print([o.shape for o in out])
